# revision 11
# baseline (speedup 1.0000x reference)
"""Trainium2 Bass kernel for nn_ConvLocalShifts.

Computes the 49 radius-3 local shifts of img [4,2,3,256,256] plus the
constant zero-padding mask, returning (shifted [4,2,3,256,256,49],
mask [4,2,1,256,256,49]) like the reference.

Sharding: pure data parallel — B*T = 8 samples, one per NeuronCore.

Per-core device program (built once, run SPMD on cores 0-7):
  input  xp [3,262,262]  f32  zero-padded sample (pad baked in on host)
  input  mr [38,262]     f32  padded 0/1 rows for this core's 32 mask rows
  output sh [3,256,12544] f32  the 49-shift interleave, [c][h][w*49+q]
  output mk [32,12544]    f32  this core's 32 rows of the shared mask

For each (channel, 128-row half): DMA 7 row-shifted staging tiles
[128,262] (block b reads padded rows h0+p+6-b), then 7 strided engine
copies build the interleaved output tile T[p, w*49 + b*7 + r] =
stage_b[p, 6 + w - r] (dest strides [49,1], src strides [1,-1]), then
one contiguous 6.4MB DMA to HBM.  q = 7b + r maps to the reference's
inverted-order shift (di=3-b, dj=3-r).  The mask chunk uses the same
machinery on the 0/1 rows.
"""

import numpy as np

import concourse.bass as bass
import concourse.bacc as bacc
import concourse.mybir as mybir
import concourse.tile as tile
from concourse.bass_utils import run_bass_kernel_spmd

B, T, C, H, W = 4, 2, 3, 256, 256
M = 8                    # cores == B*T samples
R = 3                    # local radius
KS = 2 * R + 1           # 7
K = KS * KS              # 49
PW = W + 2 * R           # 262 padded width
PH = H + 2 * R           # 262 padded height
MROWS = H // M           # 32 mask rows per core
FD = W * K               # 12544 = free dim of one interleaved row

_nc_cache = []


def _build(finalize=True):
    nc = bacc.Bacc("TRN2", target_bir_lowering=False, debug=False)
    xp = nc.dram_tensor("xp", [C, PH, PW], mybir.dt.float32, kind="ExternalInput")
    mr = nc.dram_tensor("mr", [MROWS + 2 * R, PW], mybir.dt.float32, kind="ExternalInput")
    sh = nc.dram_tensor("sh", [C, H, FD], mybir.dt.float32, kind="ExternalOutput")
    mk = nc.dram_tensor("mk", [MROWS, FD], mybir.dt.float32, kind="ExternalOutput")

    def interleave_copies(dst_tile, stage_tiles, nrows, use_vector):
        # dst[p, w*K + b*KS + r] = stage_b[p, 2R + w - r]
        # All copies for one tile go on ONE engine so the store DMA only
        # waits on a single producer sem (walrus sync-wait limit).
        for b in range(KS):
            dst = dst_tile[:nrows].rearrange("p (w k) -> p w k", k=K)[:, :, b * KS:(b + 1) * KS]
            src = bass.AP(stage_tiles[b].tensor, 2 * R, [[PW, nrows], [1, W], [-1, KS]])
            if use_vector:
                nc.vector.tensor_copy(dst, src)
            else:
                nc.scalar.copy(dst, src)

    with tile.TileContext(nc) as tc:
        with (
            tc.tile_pool(name="stage", bufs=2) as sp,
            tc.tile_pool(name="out", bufs=2) as op,
            tc.tile_pool(name="mstage", bufs=1) as msp,
            tc.tile_pool(name="mout", bufs=1) as mop,
        ):
            for c in range(C):
                for half in range(2):
                    h0 = 128 * half
                    xs = []
                    for b in range(KS):
                        t = sp.tile([128, PW], mybir.dt.float32, tag=f"xp{b}")
                        # stage_b[p,:] = padded row (h0 + p + R - b) + R
                        nc.scalar.dma_start(t[:, :], xp[c, h0 + 2 * R - b: h0 + 2 * R - b + 128, :])
                        xs.append(t)
                    To = op.tile([128, FD], mybir.dt.float32, tag="T")
                    interleave_copies(To, xs, 128, use_vector=(2 * c + half) % 2 == 0)
                    nc.sync.dma_start(sh[c, h0:h0 + 128, :], To[:, :])
            # mask chunk: 32 rows from the 0/1 row input
            ms = []
            for b in range(KS):
                t = msp.tile([MROWS, PW], mybir.dt.float32, tag=f"m{b}")
                nc.scalar.dma_start(t[:, :], mr[2 * R - b: 2 * R - b + MROWS, :])
                ms.append(t)
            Tm = mop.tile([MROWS, FD], mybir.dt.float32)
            interleave_copies(Tm, ms, MROWS, use_vector=True)
            nc.sync.dma_start(mk[:, :], Tm[:, :])
    if finalize:
        nc.finalize()
    return nc


def _get_nc():
    if not _nc_cache:
        _nc_cache.append(_build())
    return _nc_cache[0]


def _make_inputs(img):
    x8 = np.ascontiguousarray(img.reshape(M, C, H, W), dtype=np.float32)
    xpad = np.zeros((M, C, PH, PW), np.float32)
    xpad[:, :, R:R + H, R:R + W] = x8
    mrs = np.zeros((M, MROWS + 2 * R, PW), np.float32)
    for m in range(M):
        h0m = MROWS * m
        for j in range(MROWS + 2 * R):
            s = h0m - R + j
            if 0 <= s < H:
                mrs[m, j, R:R + W] = 1.0
    return xpad, mrs


def _assemble(results):
    sh = np.stack([np.asarray(r["sh"]) for r in results])
    shifted = sh.reshape(B, T, C, H, W, K)
    mk = np.concatenate([np.asarray(r["mk"]).reshape(MROWS, W, K) for r in results], axis=0)
    mask = np.ascontiguousarray(
        np.broadcast_to(mk[None, None, None], (B, T, 1, H, W, K))
    )
    return shifted, mask


def run(img, **spmd_kwargs):
    """Run the SPMD kernel; returns (BassKernelResults, (shifted, mask))."""
    img = np.asarray(img)
    assert img.shape == (B, T, C, H, W), img.shape
    xpad, mrs = _make_inputs(img)
    nc = _get_nc()
    in_maps = [{"xp": xpad[m], "mr": mrs[m]} for m in range(M)]
    res = run_bass_kernel_spmd(nc, in_maps, core_ids=list(range(M)), **spmd_kwargs)
    return res, _assemble(res.results)


def kernel(img):
    _, out = run(img)
    return out
